# revision 11
# baseline (speedup 1.0000x reference)
"""GQA kernel for 8 trn2 NeuronCores — v4.

Sharding: tensor-parallel over heads. Core c owns KV head c and Q heads
4c..4c+3 (cols 256c:256c+256 of Wq, col 64c:64c+64 of Wk/Wv, rows
256c:256c+256 of Wo). Each core computes a partial output [B,S,E]
(its ctx slice @ its Wo row-slice); host sums the 8 partials.

Device algorithm (per core, per batch):
  A. projections:
     Q.T pair tiles [128=2heads x 64d, S] (1/8 scale folded into Wq host-side)
     K.T [64, S] + duplicate to partitions 64:128 (SBUF->SBUF DMA)
     V natural [S, 64] computed directly (lhsT = x.T chunk, rhs = Wv chunk),
       token-chunk-outer (PSUM zero-region = whole bank) -> vna [128,16,65]
       with ones in col 64 (fused softmax denominator)
  B. per (q-chunk jq of 512, pair p):
       scores S.T[kv, q] row-tiled pairs -> exp on ACT -> pt bf16 [128, 1024]
       ctx NATURAL: ctx[q, 0:65] += pt_chunk.T @ V_aug  (col 64 = denom)
       normalize: per-partition reciprocal + tensor_scalar -> ctxn [128,128]
       PE-transpose (identity) ctxn -> per-(p,jq) ctxT tiles [128, 512]
  C. per jq block (emitted interleaved with B/A of next batch):
       out_partial = ctxT.T @ Wo (psum) -> DVE copy -> bf16 -> SWDGE DMA out

Emission: A0, then for jq: [B0(jq), A1-slice(jq), C0(jq)], then
for jq: [B1(jq), C1(jq)] — keeps PE/ACT/DVE/DMA queues interleaved.
"""

import numpy as np
import ml_dtypes

B = 2
S = 2048
E = 2048
HD = 64          # head dim
HPC = 4          # q heads per core
NP = 2           # head pairs per core
QD = HPC * HD    # 256 per-core q dims
NCORES = 8
EC = E // 128    # 16 contraction chunks
NJQ = S // 512   # 4 q-chunks of 512
NKV = S // 128   # 16 kv chunks of 128
BF16 = ml_dtypes.bfloat16

_cache = {}


def _build():
    from contextlib import ExitStack
    from concourse import bacc, tile
    import concourse.mybir as mybir

    bf16 = mybir.dt.bfloat16
    f32 = mybir.dt.float32
    EXP = mybir.ActivationFunctionType.Exp

    nc = bacc.Bacc(
        "TRN2", target_bir_lowering=False, debug=False, num_devices=NCORES)
    qT_d = nc.declare_dram_parameter("qT", [B, E, S], bf16, isOutput=False)
    kT_d = nc.declare_dram_parameter("kT", [B, E, S], bf16, isOutput=False)
    vT_d = nc.declare_dram_parameter("vT", [B, E, S], bf16, isOutput=False)
    wq_d = nc.declare_dram_parameter("wq", [E, QD], bf16, isOutput=False)
    wk_d = nc.declare_dram_parameter("wk", [E, HD], bf16, isOutput=False)
    wv_d = nc.declare_dram_parameter("wv", [E, HD], bf16, isOutput=False)
    wo_d = nc.declare_dram_parameter("wo", [QD, E], bf16, isOutput=False)
    id_d = nc.declare_dram_parameter("ident", [128, 128], bf16, isOutput=False)
    out_d = nc.declare_dram_parameter("out", [B, S, E], bf16, isOutput=True)

    with ExitStack() as ctx:
        tc = ctx.enter_context(tile.TileContext(nc))
        # ---- pools ----
        wpool = ctx.enter_context(tc.tile_pool(name="w", bufs=1))
        qin = ctx.enter_context(tc.tile_pool(name="qin", bufs=16))
        kvin = ctx.enter_context(tc.tile_pool(name="kvin", bufs=4))
        qts = ctx.enter_context(tc.tile_pool(name="qts", bufs=2))
        ctp = ctx.enter_context(tc.tile_pool(name="ctp", bufs=8))
        vnp = ctx.enter_context(tc.tile_pool(name="vnp", bufs=2))
        ptp = ctx.enter_context(tc.tile_pool(name="ptp", bufs=16))
        cnp = ctx.enter_context(tc.tile_pool(name="cnp", bufs=8))
        rcp = ctx.enter_context(tc.tile_pool(name="rcp", bufs=4))
        ostp = ctx.enter_context(tc.tile_pool(name="ostp", bufs=3))
        psa = ctx.enter_context(tc.tile_pool(name="psa", bufs=2, space="PSUM"))
        psx = ctx.enter_context(tc.tile_pool(name="psx", bufs=2, space="PSUM"))
        psc = ctx.enter_context(tc.tile_pool(name="psc", bufs=2, space="PSUM"))

        # ---- weights (loaded once) ----
        wq_sb = wpool.tile([128, EC, QD], bf16)
        nc.sync.dma_start(wq_sb[:], wq_d.rearrange("(c p) m -> p c m", p=128))
        wk_sb = wpool.tile([128, EC, HD], bf16)
        nc.sync.dma_start(wk_sb[:], wk_d.rearrange("(c p) m -> p c m", p=128))
        wv_sb = wpool.tile([128, EC, HD], bf16)
        nc.sync.dma_start(wv_sb[:], wv_d.rearrange("(c p) m -> p c m", p=128))
        wo_sb = wpool.tile([128, 2, E], bf16)
        nc.sync.dma_start(wo_sb[:], wo_d.rearrange("(c p) e -> p c e", p=128))
        id_sb = wpool.tile([128, 128], bf16)
        nc.sync.dma_start(id_sb[:], id_d[:, :])

        def a_load_q(b):
            qtiles = []
            for e in range(EC):
                qt = qin.tile([128, S], bf16, tag="qin", name="qt")
                nc.sync.dma_start(qt[:], qT_d[b, e * 128:(e + 1) * 128, :])
                qtiles.append(qt)
            return qtiles

        def a_qproj(b, qtiles, qp_sb, m):
            for t in range(NJQ):
                acc = psa.tile([128, 512], f32, tag="acc", name="qacc")
                for e in range(EC):
                    nc.tensor.matmul(
                        acc[:], lhsT=wq_sb[:, e, m * 128:(m + 1) * 128],
                        rhs=qtiles[e][:, t * 512:(t + 1) * 512],
                        start=(e == 0), stop=(e == EC - 1))
                nc.vector.tensor_copy(
                    qp_sb[m][:, t * 512:(t + 1) * 512], acc[:])

        def a_k(b):
            # K.T [64, S] (+dup to 64:128), streamed kT, partition-split accs
            kt2_sb = qts.tile([128, S], bf16, tag="kt2")
            kaccs = [psa.tile([128, 512], f32, tag="acc", name="kacc")
                     for _ in range(2)]
            for e in range(EC):
                kt_in = kvin.tile([128, S], bf16, tag="kvin", name="ktin")
                nc.sync.dma_start(kt_in[:], kT_d[b, e * 128:(e + 1) * 128, :])
                for t in range(4):
                    r0 = (t % 2) * 64
                    nc.tensor.matmul(
                        kaccs[t // 2][r0:r0 + 64, :], lhsT=wk_sb[:, e, :],
                        rhs=kt_in[:, t * 512:(t + 1) * 512],
                        start=(e == 0), stop=(e == EC - 1),
                        tile_position=(0, r0))
            for t in range(4):
                r0 = (t % 2) * 64
                nc.vector.tensor_copy(
                    kt2_sb[0:64, t * 512:(t + 1) * 512],
                    kaccs[t // 2][r0:r0 + 64, :])
            # dup on the ACT hwdge queue: keeps its wait off SP.SEQ so the
            # vT loads behind it are not head-of-line blocked
            nc.scalar.dma_start(kt2_sb[64:128, :], kt2_sb[0:64, :])
            return kt2_sb

        def a_v(b):
            # V natural [S, 64] + ones col -> vna [128,16,65]
            # accumulation regions sharing a PSUM bank must run start..stop
            # strictly sequentially (start zeroes the whole 2KB bank region)
            # -> token-chunk-outer loop, e-inner.
            vna = vnp.tile([128, NKV, HD + 1], bf16, tag="vna", name="vna")
            nc.vector.memset(vna[:, :, HD:HD + 1], 1.0)
            for half in range(2):
                vts = []
                for e in range(EC):
                    vt = qin.tile([128, S // 2], bf16, tag="qin", name="vt")
                    nc.sync.dma_start(
                        vt[:], vT_d[b, e * 128:(e + 1) * 128,
                                    half * 1024:(half + 1) * 1024])
                    vts.append(vt)
                for t8 in range(8):
                    t = half * 8 + t8
                    vacc = psa.tile([128, 64], f32, tag="acc", name="vacc")
                    for e in range(EC):
                        nc.tensor.matmul(
                            vacc[:, 0:64],
                            lhsT=vts[e][:, t8 * 128:(t8 + 1) * 128],
                            rhs=wv_sb[:, e, :],
                            start=(e == 0), stop=(e == EC - 1))
                    nc.vector.tensor_copy(vna[:, t, 0:HD], vacc[:, 0:64])
            return vna

        def b_scores(b, jq, p, qp_sb, kt2_sb):
            """Scores + exp for one (q-window, pair) block."""
            pts = [[None] * (NKV // 2) for _ in range(2)]
            for g in range(NKV // 2):
                sc_e = psc.tile([128, 1024], f32, tag="sc", name="sc_e")
                sc_o = psc.tile([128, 1024], f32, tag="sc", name="sc_o")
                for ki in range(2):
                    kv = g * 2 + ki
                    nc.tensor.matmul(
                        sc_e[:, ki * 512:(ki + 1) * 512],
                        lhsT=kt2_sb[0:64, kv * 128:(kv + 1) * 128],
                        rhs=qp_sb[p][0:64, jq * 512:(jq + 1) * 512],
                        start=True, stop=True)
                    nc.tensor.matmul(
                        sc_o[:, ki * 512:(ki + 1) * 512],
                        lhsT=kt2_sb[64:128, kv * 128:(kv + 1) * 128],
                        rhs=qp_sb[p][64:128, jq * 512:(jq + 1) * 512],
                        start=True, stop=True)
                pt_e = ptp.tile([128, 1024], bf16, tag="pt", name="pt_e")
                nc.scalar.activation(pt_e[:], sc_e[:], EXP)
                pt_o = ptp.tile([128, 1024], bf16, tag="pt", name="pt_o")
                nc.scalar.activation(pt_o[:], sc_o[:], EXP)
                pts[0][g] = pt_e
                pts[1][g] = pt_o
            return pts

        def b_ctx(b, jq, p, pts, vna, ct):
            """ctx natural + normalize + transpose for one block."""
            ctxn = [cnp.tile([128, 128], bf16, tag="ctxn", name="ctxn")
                    for _ in range(4)]
            for h in range(2):
                cps = psx.tile([128, 4 * 65], f32, tag="cps", name="cps")
                for j in range(4):
                    for kv in range(NKV):
                        g, ki = kv // 2, kv % 2
                        nc.tensor.matmul(
                            cps[:, j * 65:(j + 1) * 65],
                            lhsT=pts[h][g][:, ki * 512 + j * 128:
                                           ki * 512 + (j + 1) * 128],
                            rhs=vna[:, kv, :],
                            start=(kv == 0), stop=(kv == NKV - 1))
                rc = rcp.tile([128, 4], f32, tag="rc", name="rc")
                for j in range(4):
                    nc.vector.reciprocal(
                        rc[:, j:j + 1],
                        cps[:, j * 65 + HD:j * 65 + HD + 1])
                for j in range(4):
                    nc.vector.tensor_scalar_mul(
                        ctxn[j][:, h * 64:(h + 1) * 64],
                        cps[:, j * 65:j * 65 + HD], rc[:, j:j + 1])
            for j in range(4):
                tp = psx.tile([128, 128], bf16, tag="cps", name="tp")
                nc.tensor.transpose(tp[:], ctxn[j][:], id_sb[:])
                nc.vector.tensor_copy(ct[:, j * 128:(j + 1) * 128], tp[:])

        def phase_B(b, qp_sb, kt2_sb, vna, c_emit=None, pre=None):
            """Blocks streamed with scores/exp one block AHEAD of ctx, so
            score matmuls always outrank ctx in scheduler priority and the
            ACT exp pipeline never starves at block boundaries. `pre` holds
            already-emitted (pts, ct) for the first blocks (hoisted ahead of
            the V projection so exp starts as early as possible)."""
            cts = {}
            pend = None
            for jq in range(NJQ):
                for p in range(NP):
                    if pre is not None and jq == 0 and p < len(pre):
                        pts, ct = pre[p]
                    else:
                        pts = b_scores(b, jq, p, qp_sb, kt2_sb)
                        ct = ctp.tile([128, 512], bf16, tag=f"ctxT{p}",
                                      name=f"ctxT{p}")
                    cts.setdefault(jq, [None, None])[p] = ct
                    if pend is not None:
                        pj, pp, ppts, pct = pend
                        b_ctx(b, pj, pp, ppts, vna, pct)
                        if pp == 1 and c_emit is not None:
                            c_emit(pj, cts[pj])
                    pend = (jq, p, pts, ct)
            pj, pp, ppts, pct = pend
            b_ctx(b, pj, pp, ppts, vna, pct)
            if c_emit is not None:
                c_emit(pj, cts[pj])
            return cts

        def c_jq(b, ctiles, jq):
            """Output projection for the 4 token-chunks of one q-window.
            Out DMA goes through gpsimd SWDGE (Pool engine is idle)."""
            for t4 in range(4):
                t = jq * 4 + t4
                ost = ostp.tile([128, E], bf16, tag="ost", name="ost")
                for eh in range(4):
                    ops = psa.tile([128, 512], f32, tag="acc", name="ops")
                    for kc in range(2):
                        nc.tensor.matmul(
                            ops[:],
                            lhsT=ctiles[kc][:, t4 * 128:(t4 + 1) * 128],
                            rhs=wo_sb[:, kc, eh * 512:(eh + 1) * 512],
                            start=(kc == 0), stop=(kc == 1))
                    nc.vector.tensor_copy(ost[:, eh * 512:(eh + 1) * 512],
                                          ops[:])
                nc.gpsimd.dma_start(
                    out_d[b, t * 128:(t + 1) * 128, :], ost[:])

        # ---- software-pipelined emission ----
        qt0 = a_load_q(0)
        qp0 = [qts.tile([128, S], bf16, tag=f"qp{p}", name=f"qp{p}")
               for p in range(NP)]
        a_qproj(0, qt0, qp0, 0)
        a_qproj(0, qt0, qp0, 1)
        kt20 = a_k(0)
        # hoist the first two blocks' scores/exp ahead of the V projection:
        # they only need Q+K, so the ACT exp pipeline starts ~20us earlier
        pre0 = []
        for p in range(NP):
            pts = b_scores(0, 0, p, qp0, kt20)
            ct = ctp.tile([128, 512], bf16, tag=f"ctxT{p}", name=f"ctxT{p}")
            pre0.append((pts, ct))
        vna0 = a_v(0)

        # B0 emitted alone so its score matmuls keep top scheduler priority
        # (ACT exp is the co-bottleneck; PE is oversubscribed during B0).
        cts0 = phase_B(0, qp0, kt20, vna0, pre=pre0)
        # A1 is gap-filler for B0's ACT-bound stretches.
        qt1 = a_load_q(1)
        qp1 = [qts.tile([128, S], bf16, tag=f"qp{p}", name=f"qp{p}")
               for p in range(NP)]
        a_qproj(1, qt1, qp1, 0)
        a_qproj(1, qt1, qp1, 1)
        kt21 = a_k(1)
        vna1 = a_v(1)
        # C0 runs below everything else (pure gap-filler): B1's scores must
        # outrank it or the exp pipeline starves at the batch handoff.
        with tc.high_priority(offset=-200000):
            for jq in range(NJQ):
                c_jq(0, cts0[jq], jq)
        # B1 has PE slack; inline C1 per jq to shrink the serial tail.
        phase_B(1, qp1, kt21, vna1,
                c_emit=lambda jq, ct: c_jq(1, ct, jq))
    nc.compile()
    return nc


def _get_nc():
    if "nc" not in _cache:
        _cache["nc"] = _build()
    return _cache["nc"]


def kernel(query, key, value, Wq, Wk, Wv, Wo, _trace=False):
    from concourse.bass_utils import run_bass_kernel_spmd

    def t_bf16(x):
        return np.ascontiguousarray(
            np.asarray(x, np.float32).astype(BF16).transpose(0, 2, 1))

    qT = t_bf16(query)
    kT = t_bf16(key)
    vT = t_bf16(value)
    # fold 1/sqrt(HD) into Wq
    Wq = (np.asarray(Wq, np.float32) * 0.125).astype(BF16)
    Wk = np.asarray(Wk, np.float32).astype(BF16)
    Wv = np.asarray(Wv, np.float32).astype(BF16)
    Wo = np.asarray(Wo, np.float32).astype(BF16)
    ident = np.eye(128, dtype=BF16)

    in_maps = []
    for c in range(NCORES):
        in_maps.append({
            "qT": qT, "kT": kT, "vT": vT,
            "wq": np.ascontiguousarray(Wq[:, c * QD:(c + 1) * QD]),
            "wk": np.ascontiguousarray(Wk[:, c * HD:(c + 1) * HD]),
            "wv": np.ascontiguousarray(Wv[:, c * HD:(c + 1) * HD]),
            "wo": np.ascontiguousarray(Wo[c * QD:(c + 1) * QD, :]),
            "ident": ident,
        })

    nc = _get_nc()
    res = run_bass_kernel_spmd(nc, in_maps, list(range(NCORES)), trace=_trace)
    out = res.results[0]["out"].astype(np.float32)
    for c in range(1, NCORES):
        out += res.results[c]["out"].astype(np.float32)
    if _trace:
        _cache["last_exec_time_ns"] = res.exec_time_ns
        _cache["last_results"] = res
    return out
